# revision 16
# baseline (speedup 1.0000x reference)
"""MultiHeadAttention with RoPE on 8 Trainium2 NeuronCores (Bass/Tile).

Sharding: batch x head-group. Core c handles batch b = c//2 and head
group g = c%2 (8 of 16 heads, i.e. a 512-wide slice of the model dim).
Each core computes Q/K/V projections for its slice, RoPE, attention for
its 8 heads, and a partial output projection (row-parallel TP). The host
sums the two partials per batch and adds the output bias.

Device layouts (per core):
  xT (q/k/v inputs, transposed on host): [D=1024, S=2048] bf16
  QT/KT  = W x.T (+b, RoPE)            : [512(j), 2048(s)] bf16, j on partitions
  V_aug                                : [2048(s), 8*65] bf16 (per head: 64 V
                                         cols + a ones column that makes the
                                         P.T @ V_aug matmul also produce the
                                         softmax row sums)
  S.T    = KT.T-slices @ QT            : PSUM [128(k), 1024(q)] per k-tile
  P.T    = exp(S.T / 8)                : bf16 tiles (no max subtraction:
                                         scores are O(1) by construction)
  O.T    = V_aug.T @ P.T               : PSUM [65, 1024], row 64 = sums
  out    = O.T-tiles.T @ WoT           : [2048, 1024] fp32 partial
"""

import os
import sys

for _p in ("/opt/trn_rl_repo", os.path.expanduser("~/.axon_site/_ro/trn_rl_repo")):
    if os.path.isdir(_p) and _p not in sys.path:
        sys.path.insert(0, _p)
        break

import numpy as np
import ml_dtypes

import concourse.bass as bass
import concourse.mybir as mybir
from concourse.bass_utils import run_bass_kernel_spmd
from concourse.tile import TileContext
from concourse.vector_clock import ScopedClock

BF16 = ml_dtypes.bfloat16
FP32 = mybir.dt.float32
BF = mybir.dt.bfloat16

B, S, D = 4, 2048, 1024
H, HD = 16, 64
HL = 8          # heads per core
JL = HL * HD    # 512 local j-dims
N_CORES = 8
PD = 128        # partition dim
DC = D // PD    # 8 d-chunks
NJT = JL // PD  # 4 j-tiles (head pairs)
NST = S // PD   # 16 s-tiles
SC = 512        # matmul free-dim chunk
NSC = S // SC   # 4 s-chunks
QH = 1024       # q processed in halves
VW = HD + 1     # 65: V columns + ones column

AluOp = mybir.AluOpType
ActFn = mybir.ActivationFunctionType

# ---------------------------------------------------------------------------
# Workaround for this walrus build: instructions carrying >1 semaphore wait
# fail codegen ("Too many sync wait commands"). Split them onto nops.
# ---------------------------------------------------------------------------
_MAX_WAITS = 1


def _patched_drain_and_barrier(self, tick_clock, wait_clock):
    nc = self.nc
    probe = nc.sync.nop()
    wait_clock.add_sem_waits(probe.ins, ScopedClock({None: tick_clock.global_clock}))
    si = probe.ins.sync_info
    waits = list(si.on_wait) if si and si.on_wait else []
    upds = list(si.on_update) if si and si.on_update else []
    probe.ins.sync_info = mybir.SyncInfo(on_wait=waits[:_MAX_WAITS], on_update=upds)
    for w in waits[_MAX_WAITS:]:
        n = nc.sync.nop()
        n.ins.sync_info = mybir.SyncInfo(on_wait=[w], on_update=[])
    nc.sync.drain()
    nc.all_engine_barrier()
    assert self.sems is not None
    popped = nc._tile_sem_poison_stack.pop()
    assert popped is self._sem_poison
    nc.clear_and_free_semaphores(list(self.sems.allocated().values()))
    nc.all_engine_barrier()


TileContext._drain_and_barrier = _patched_drain_and_barrier


def _split_multiwaits(nc: bass.Bass) -> int:
    n_split = 0
    for f in nc.m.functions:
        for b in f.blocks:
            insts = b.instructions
            if not any(
                i.sync_info and i.sync_info.on_wait and len(i.sync_info.on_wait) > _MAX_WAITS
                for i in insts
            ):
                continue
            new_list = []
            for inst in insts:
                si = inst.sync_info
                if si and si.on_wait and len(si.on_wait) > _MAX_WAITS:
                    waits = list(si.on_wait)
                    for w in waits[_MAX_WAITS:]:
                        nop = nc.engines[inst.engine].nop()
                        cur = nc.cur_bb.bb
                        tail = cur.instructions
                        assert tail[-1].name == nop.ins.name
                        cur.instructions = tail[:-1]
                        nop.ins.sync_info = mybir.SyncInfo(on_wait=[w], on_update=[])
                        new_list.append(nop.ins)
                        n_split += 1
                    inst.sync_info = mybir.SyncInfo(
                        on_wait=waits[:_MAX_WAITS],
                        on_update=list(si.on_update) if si.on_update else [],
                    )
                new_list.append(inst)
            b.instructions = new_list
    return n_split


# ---------------------------------------------------------------------------
# Program builder
# ---------------------------------------------------------------------------

def build_program(n_pairs=NJT, with_attn=True, with_outproj=True) -> bass.Bass:
    nc = bass.Bass()

    xq = nc.declare_dram_parameter("xq_t", [D, S], BF, isOutput=False)
    xk = nc.declare_dram_parameter("xk_t", [D, S], BF, isOutput=False)
    xv = nc.declare_dram_parameter("xv_t", [D, S], BF, isOutput=False)
    wq = nc.declare_dram_parameter("wq_t", [D, JL], BF, isOutput=False)
    wk = nc.declare_dram_parameter("wk_t", [D, JL], BF, isOutput=False)
    wv = nc.declare_dram_parameter("wv_t", [D, JL], BF, isOutput=False)
    wo = nc.declare_dram_parameter("wo_t", [JL, D], BF, isOutput=False)
    bq = nc.declare_dram_parameter("bq", [JL, 1], FP32, isOutput=False)
    bk = nc.declare_dram_parameter("bk", [JL, 1], FP32, isOutput=False)
    bv_map = nc.declare_dram_parameter("bv_map", [PD, JL], FP32, isOutput=False)
    cc_in = nc.declare_dram_parameter("cc", [PD, S], FP32, isOutput=False)
    ss_in = nc.declare_dram_parameter("ss", [PD, S], FP32, isOutput=False)
    out_p = nc.declare_dram_parameter("out_p", [S, D], FP32, isOutput=True)

    with TileContext(nc) as tc:
        with (
            tc.tile_pool(name="const", bufs=1) as constp,
            tc.tile_pool(name="xt", bufs=16) as xtp,
            tc.tile_pool(name="qkv", bufs=1) as qkvp,
            tc.tile_pool(name="psA", bufs=1, space="PSUM") as psA,
            tc.tile_pool(name="rope", bufs=2) as ropep,
            tc.tile_pool(name="st", bufs=2, space="PSUM") as stp,
            tc.tile_pool(name="ot", bufs=1, space="PSUM") as otp,
            tc.tile_pool(name="pt", bufs=2) as ptp,
            tc.tile_pool(name="osc", bufs=2) as oscp,
            tc.tile_pool(name="rmap", bufs=2) as rmapp,
            tc.tile_pool(name="dram", bufs=4, space="DRAM") as dramp,
            tc.tile_pool(name="outp", bufs=2) as outp,
        ):
            # ---- constant loads ----
            def load_w(dram, name):
                ts = []
                for d in range(DC):
                    t = constp.tile([PD, JL], BF, tag=f"{name}{d}")
                    nc.sync.dma_start(out=t[:], in_=dram[d * PD:(d + 1) * PD, :])
                    ts.append(t)
                return ts

            wv_t = load_w(wv, "wv")
            wq_t = load_w(wq, "wq")
            wk_t = load_w(wk, "wk")
            cc = constp.tile([PD, S], FP32, tag="cc")
            nc.sync.dma_start(out=cc[:], in_=cc_in[:])
            ss = constp.tile([PD, S], FP32, tag="ss")
            nc.sync.dma_start(out=ss[:], in_=ss_in[:])
            bvm = constp.tile([PD, JL], FP32, tag="bvm")
            nc.sync.dma_start(out=bvm[:], in_=bv_map[:])
            bq_t, bk_t = [], []
            for j in range(NJT):
                t = constp.tile([PD, 1], FP32, tag=f"bq{j}")
                nc.sync.dma_start(out=t[:], in_=bq[j * PD:(j + 1) * PD, :])
                bq_t.append(t)
                t = constp.tile([PD, 1], FP32, tag=f"bk{j}")
                nc.sync.dma_start(out=t[:], in_=bk[j * PD:(j + 1) * PD, :])
                bk_t.append(t)

            def load_xt(dram, name):
                ts = []
                for d in range(DC):
                    t = xtp.tile([PD, S], BF, tag="xt")
                    nc.sync.dma_start(out=t[:], in_=dram[d * PD:(d + 1) * PD, :])
                    ts.append(t)
                return ts

            # ---- V projection: V_aug [s, 8*65] ----
            xv_t = load_xt(xv, "xv")
            v_aug = [qkvp.tile([PD, HL * VW], BF, tag=f"vaug{st}", name=f"vaug{st}") for st in range(NST)]
            for st in range(NST):
                ones_cols = v_aug[st][:].rearrange("p (h w) -> p h w", w=VW)[:, :, HD:VW]
                nc.vector.memset(ones_cols, 1.0)
                ps = psA.tile([PD, JL], FP32, tag="vps")
                for d in range(DC):
                    nc.tensor.matmul(
                        ps[:],
                        xv_t[d][:, st * PD:(st + 1) * PD],
                        wv_t[d][:],
                        start=(d == 0),
                        stop=(d == DC - 1),
                    )
                # evac + bias into the strided V_aug slice (skip ones cols)
                dst = v_aug[st][:].rearrange("p (h w) -> p h w", w=VW)[:, :, 0:HD]
                nc.vector.tensor_tensor(
                    out=dst,
                    in0=ps[:].rearrange("p (h w) -> p h w", w=HD),
                    in1=bvm[:].rearrange("p (h w) -> p h w", w=HD),
                    op=AluOp.add,
                )

            # ---- Q/K projections with RoPE, then attention per head pair ----
            qt = [qkvp.tile([PD, S], BF, tag=f"qt{j}", name=f"qt{j}") for j in range(NJT)]
            kt = [qkvp.tile([PD, S], BF, tag=f"kt{j}", name=f"kt{j}") for j in range(NJT)]
            otb = [qkvp.tile([PD, S], BF, tag=f"otb{j}", name=f"otb{j}") for j in range(NJT)]

            xq_t = load_xt(xq, "xq")
            xk_t = load_xt(xk, "xk")

            def proj_rope(jt, x_t, w_t, b_t, out_t):
                for sc in range(NSC):
                    ps = psA.tile([PD, SC], FP32, tag="qkps")
                    for d in range(DC):
                        nc.tensor.matmul(
                            ps[:],
                            w_t[d][:, jt * PD:(jt + 1) * PD],
                            x_t[d][:, sc * SC:(sc + 1) * SC],
                            start=(d == 0),
                            stop=(d == DC - 1),
                        )
                    # bias (per-partition) in place, then rotate-half
                    nc.vector.tensor_scalar(
                        out=ps[:], in0=ps[:], scalar1=b_t[jt][:], scalar2=None,
                        op0=AluOp.add,
                    )
                    tmp = ropep.tile([PD, SC], FP32, tag="swp")
                    for h0 in (0, HD):
                        nc.vector.tensor_copy(
                            out=tmp[h0:h0 + 32, :], in_=ps[h0 + 32:h0 + 64, :]
                        )
                        nc.vector.tensor_copy(
                            out=tmp[h0 + 32:h0 + 64, :], in_=ps[h0:h0 + 32, :]
                        )
                    t1 = ropep.tile([PD, SC], FP32, tag="t1")
                    nc.vector.tensor_tensor(
                        out=t1[:], in0=ps[:], in1=cc[:, sc * SC:(sc + 1) * SC],
                        op=AluOp.mult,
                    )
                    nc.vector.tensor_tensor(
                        out=tmp[:], in0=tmp[:], in1=ss[:, sc * SC:(sc + 1) * SC],
                        op=AluOp.mult,
                    )
                    nc.vector.tensor_tensor(
                        out=out_t[:, sc * SC:(sc + 1) * SC], in0=t1[:], in1=tmp[:],
                        op=AluOp.add,
                    )

            for jt in range(n_pairs):
                proj_rope(jt, xq_t, wq_t, bq_t, qt[jt])
                proj_rope(jt, xk_t, wk_t, bk_t, kt[jt])

                # ---- attention for head pair jt (heads 2jt, 2jt+1) ----
                # 512-wide q chunks; both heads' score chunks packed into
                # one [128, 1024] PSUM tile so a single ACT exp covers both.
                for qc in range(NSC if with_attn else 0):
                    q0 = qc * SC
                    o_ps = [otp.tile([VW, SC], FP32, tag=f"ops{ab}", name=f"ops{ab}") for ab in range(2)]
                    for ktile in range(NST):
                        k0 = ktile * PD
                        st_ps = stp.tile([PD, 2 * SC], FP32, tag="st")
                        for ab in range(2):
                            hb = ab * HD
                            nc.tensor.matmul(
                                st_ps[:, ab * SC:(ab + 1) * SC],
                                kt[jt][hb:hb + HD, k0:k0 + PD],
                                qt[jt][hb:hb + HD, q0:q0 + SC],
                                start=True,
                                stop=True,
                            )
                        p_sb = ptp.tile([PD, 2 * SC], BF, tag="pt")
                        nc.scalar.activation(
                            out=p_sb[:], in_=st_ps[:], func=ActFn.Exp,
                            scale=0.125,
                        )
                        for ab in range(2):
                            h = 2 * jt + ab
                            nc.tensor.matmul(
                                o_ps[ab][:],
                                v_aug[ktile][:, h * VW:h * VW + VW],
                                p_sb[:, ab * SC:(ab + 1) * SC],
                                start=(ktile == 0),
                                stop=(ktile == NST - 1),
                            )
                    # sums + reciprocal + normalization maps
                    recs = []
                    for ab in range(2):
                        rec = rmapp.tile([1, SC], FP32, tag=f"rec{ab}", name=f"rec{ab}")
                        nc.vector.reciprocal(rec[:], o_ps[ab][HD:VW, :])
                        recs.append(rec)
                    drec = dramp.tile([2, SC], FP32, tag="drec")
                    for ab in range(2):
                        nc.sync.dma_start(out=drec[ab:ab + 1, :], in_=recs[ab][:])
                    rmap = rmapp.tile([PD, SC], FP32, tag="rmap")
                    for ab in range(2):
                        nc.sync.dma_start(
                            out=rmap[ab * HD:(ab + 1) * HD, :],
                            in_=drec[ab:ab + 1, :].partition_broadcast(HD),
                        )
                    # copy O out of PSUM, then normalize into bf16 OT.
                    # osc rows are placed at ab*64 so the multiply's two SBUF
                    # inputs share a base partition (walrus requirement).
                    osc = oscp.tile([PD, SC], FP32, tag="osc")
                    for ab in range(2):
                        nc.vector.tensor_copy(
                            out=osc[ab * HD:(ab + 1) * HD, :], in_=o_ps[ab][0:HD, :]
                        )
                        nc.vector.tensor_tensor(
                            out=otb[jt][ab * HD:(ab + 1) * HD, q0:q0 + SC],
                            in0=osc[ab * HD:(ab + 1) * HD, :],
                            in1=rmap[ab * HD:(ab + 1) * HD, :],
                            op=AluOp.mult,
                        )

            # ---- output projection (partial, row-parallel) ----
            wo_t = []
            for j in range(NJT):
                t = xtp.tile([PD, D], BF, tag="xt", name=f"wo{j}")
                nc.sync.dma_start(out=t[:], in_=wo[j * PD:(j + 1) * PD, :])
                wo_t.append(t)
            for st in range(NST if with_outproj else 0):
                s0 = st * PD
                for c in range(D // SC):
                    ps = psA.tile([PD, SC], FP32, tag="vps")
                    for jt in range(NJT):
                        nc.tensor.matmul(
                            ps[:],
                            otb[jt][:, s0:s0 + PD],
                            wo_t[jt][:, c * SC:(c + 1) * SC],
                            start=(jt == 0),
                            stop=(jt == NJT - 1),
                        )
                    o_sb = outp.tile([PD, SC], FP32, tag="osb")
                    nc.vector.tensor_copy(out=o_sb[:], in_=ps[:])
                    nc.sync.dma_start(
                        out=out_p[s0:s0 + PD, c * SC:(c + 1) * SC], in_=o_sb[:]
                    )

    _split_multiwaits(nc)
    return nc


_NC_CACHE = None


def _get_program():
    global _NC_CACHE
    if _NC_CACHE is None:
        _NC_CACHE = build_program()
    return _NC_CACHE


# ---------------------------------------------------------------------------
# Host-side sharding / gathering
# ---------------------------------------------------------------------------

def _rope_maps():
    hd = np.arange(PD) % HD
    m = hd % 32
    inv = (10000.0 ** (-(2.0 * m) / HD)).astype(np.float64)
    s = np.arange(S, dtype=np.float64)
    ang = inv[:, None] * s[None, :]
    cc = np.cos(ang)
    sign = np.where(hd < 32, -1.0, 1.0)
    ss = np.sin(ang) * sign[:, None]
    return cc.astype(np.float32), ss.astype(np.float32)


def shard_inputs(query, key, value, Wq, bq, Wk, bk, Wv, bv, Wo, bo):
    cc, ss = _rope_maps()
    in_maps = []
    for c in range(N_CORES):
        b, g = divmod(c, 2)
        jsl = slice(g * JL, (g + 1) * JL)
        m = {
            "xq_t": np.ascontiguousarray(query[b].T).astype(BF16),
            "xk_t": np.ascontiguousarray(key[b].T).astype(BF16),
            "xv_t": np.ascontiguousarray(value[b].T).astype(BF16),
            "wq_t": np.ascontiguousarray(Wq[jsl, :].T).astype(BF16),
            "wk_t": np.ascontiguousarray(Wk[jsl, :].T).astype(BF16),
            "wv_t": np.ascontiguousarray(Wv[jsl, :].T).astype(BF16),
            "wo_t": np.ascontiguousarray(Wo[:, jsl].T).astype(BF16),
            "bq": np.ascontiguousarray(bq[jsl, None]).astype(np.float32),
            "bk": np.ascontiguousarray(bk[jsl, None]).astype(np.float32),
            "bv_map": np.broadcast_to(bv[jsl], (PD, JL)).astype(np.float32).copy(),
            "cc": cc,
            "ss": ss,
        }
        in_maps.append(m)
    return in_maps


def kernel(query, key, value, Wq, bq, Wk, bk, Wv, bv, Wo, bo):
    query = np.asarray(query, dtype=np.float32)
    key = np.asarray(key, dtype=np.float32)
    value = np.asarray(value, dtype=np.float32)
    Wq, bq = np.asarray(Wq, np.float32), np.asarray(bq, np.float32)
    Wk, bk = np.asarray(Wk, np.float32), np.asarray(bk, np.float32)
    Wv, bv = np.asarray(Wv, np.float32), np.asarray(bv, np.float32)
    Wo, bo = np.asarray(Wo, np.float32), np.asarray(bo, np.float32)

    nc = _get_program()
    in_maps = shard_inputs(query, key, value, Wq, bq, Wk, bk, Wv, bv, Wo, bo)
    res = run_bass_kernel_spmd(nc, in_maps, list(range(N_CORES)))

    out = np.empty((B, S, D), np.float32)
    for b in range(B):
        out[b] = res.results[2 * b]["out_p"] + res.results[2 * b + 1]["out_p"] + bo
    return out
